# revision 6
# baseline (speedup 1.0000x reference)
"""GPT forward (L=3, D=1024, H=16, T=1024, B=4, V=32000) on 8 trn2 cores.

Sharding: attention head-parallel (2 heads/core); proj+FFN+LN
token-parallel (512 tokens/core); LM head vocab-parallel (4000/core).
Collectives per layer: AllGather(xn^T, 2.1MB) before QKV and
AllToAll(o^T, 2.1MB) before the proj (A2A hands each core exactly
oT_full[:, my_tokens] at a static address). One final AllGather(xf^T)
before the LM head. Activations are kept feature-major for GEMMs via
128x128 PE transposes; the residual stream stays token-major per core.
Embedding lookup and the loss reduction run on host.
"""
import sys

for _p in ("/opt/trn_rl_repo",):
    if _p not in sys.path:
        sys.path.insert(0, _p)

import numpy as np

import concourse.bass as bass
import concourse.mybir as mybir
import concourse.tile as tile
from concourse import bacc
from concourse.bass_utils import run_bass_kernel_spmd
from concourse.masks import make_causal_mask, make_identity

F32 = mybir.dt.float32
AX = mybir.AxisListType.X
ALU = mybir.AluOpType
ACTF = mybir.ActivationFunctionType

NC = 8          # cores
V, BLK, H, D, L = 32000, 1024, 16, 1024, 3
HD = D // H     # 64
B, T = 4, 1024
TT = B * T      # 4096 tokens total
TS = TT // NC   # 512 tokens per core
NTM = TS // 128  # 4 token tiles per core
KD = D // 128    # 8 feature chunks
HL = H // NC     # 2 local heads
FF = 4 * D       # 4096
KF = FF // 128   # 32 hidden chunks
VC = V // NC     # 4000 vocab per core
NQ = T // 128    # 8 q tiles per batch
SCALE = float(D) ** -0.5
MASKVAL = -1e9
EPS = 1e-5

_CACHE = {}


def _bcast(ap_row, parts=128):
    """row AP [1, n] -> [parts, n] partition-broadcast (stride-0) AP."""
    return bass.AP(tensor=ap_row.tensor, offset=ap_row.offset,
                   ap=[[0, parts]] + [list(ap_row.ap[-1])])


def build_program():
    if "nc" in _CACHE:
        return _CACHE["nc"]
    nc = bacc.Bacc("TRN2", target_bir_lowering=False, debug=False,
                   enable_asserts=True, num_devices=NC)

    x0 = nc.dram_tensor("x0", [TS, D], F32, kind="ExternalInput")
    wqkv = nc.dram_tensor("wqkv", [L, D, 3 * 2 * HD], F32, kind="ExternalInput")
    wproj = nc.dram_tensor("wproj", [L, D, D], F32, kind="ExternalInput")
    bproj = nc.dram_tensor("bproj", [L, D], F32, kind="ExternalInput")
    ln1g = nc.dram_tensor("ln1g", [L, D], F32, kind="ExternalInput")
    ln1b = nc.dram_tensor("ln1b", [L, D], F32, kind="ExternalInput")
    ln2g = nc.dram_tensor("ln2g", [L, D], F32, kind="ExternalInput")
    ln2b = nc.dram_tensor("ln2b", [L, D], F32, kind="ExternalInput")
    w1 = nc.dram_tensor("w1", [L, D, FF], F32, kind="ExternalInput")
    b1 = nc.dram_tensor("b1", [L, FF], F32, kind="ExternalInput")
    w2 = nc.dram_tensor("w2", [L, FF, D], F32, kind="ExternalInput")
    b2 = nc.dram_tensor("b2", [L, D], F32, kind="ExternalInput")
    lnfg = nc.dram_tensor("lnfg", [1, D], F32, kind="ExternalInput")
    lnfb = nc.dram_tensor("lnfb", [1, D], F32, kind="ExternalInput")
    wlm = nc.dram_tensor("wlm", [D, VC], F32, kind="ExternalInput")
    blm = nc.dram_tensor("blm", [1, VC], F32, kind="ExternalInput")
    logits = nc.dram_tensor("logits", [TT, VC], F32, kind="ExternalOutput")

    with tile.TileContext(nc) as tc:
        _emit(nc, tc, x0, wqkv, wproj, bproj, ln1g, ln1b, ln2g, ln2b,
              w1, b1, w2, b2, lnfg, lnfb, wlm, blm, logits)
    nc.compile()
    _CACHE["nc"] = nc
    return nc


def _emit(nc, tc, x0, wqkv, wproj, bproj, ln1g, ln1b, ln2g, ln2b,
          w1, b1, w2, b2, lnfg, lnfb, wlm, blm, logits):
    from contextlib import ExitStack
    with ExitStack() as ctx:
        const = ctx.enter_context(tc.tile_pool(name="const", bufs=1))
        identity = const.tile([128, 128], F32)
        make_identity(nc, identity)
        mask128 = const.tile([128, 128], F32)
        make_causal_mask(nc, mask128, mask_val=MASKVAL)
        eps_t = const.tile([128, 1], F32)
        nc.vector.memset(eps_t, EPS)

        persist = ctx.enter_context(tc.tile_pool(name="persist", bufs=1))
        x_my = persist.tile([128, NTM, D], F32)  # residual, token-major
        nc.sync.dma_start(out=x_my, in_=x0.ap().rearrange("(t p) d -> p t d", p=128))

        dram = ctx.enter_context(tc.tile_pool(name="dram", bufs=1, space="DRAM"))
        ln_sc = ctx.enter_context(tc.tile_pool(name="ln_sc", bufs=4))
        bc_pool = ctx.enter_context(tc.tile_pool(name="bc", bufs=2))
        xn_pool = ctx.enter_context(tc.tile_pool(name="xn", bufs=1))
        xnT_pool = ctx.enter_context(tc.tile_pool(name="xnT", bufs=1))

        def layer_norm(src3d, g_row, b_row, dst3d):
            g_bc = bc_pool.tile([128, D], F32, tag="g_bc")
            b_bc = bc_pool.tile([128, D], F32, tag="b_bc")
            nc.sync.dma_start(out=g_bc, in_=_bcast(g_row))
            nc.sync.dma_start(out=b_bc, in_=_bcast(b_row))
            for m in range(NTM):
                xt = src3d[:, m, :]
                stats = ln_sc.tile([128, 2, 6], F32, tag="stats")
                nc.vector.bn_stats(out=stats[:, 0, :], in_=xt[:, 0:512])
                nc.vector.bn_stats(out=stats[:, 1, :], in_=xt[:, 512:1024])
                mv = ln_sc.tile([128, 2], F32, tag="mv")
                nc.vector.bn_aggr(out=mv, in_=stats)
                sd = ln_sc.tile([128, 1], F32, tag="sd")
                nc.scalar.activation(out=sd, in_=mv[:, 1:2], func=ACTF.Sqrt,
                                     bias=eps_t, scale=1.0)
                ri = ln_sc.tile([128, 1], F32, tag="ri")
                nc.vector.reciprocal(out=ri, in_=sd)
                nm = ln_sc.tile([128, 1], F32, tag="nm")
                nc.vector.tensor_scalar(out=nm, in0=mv[:, 0:1], scalar1=ri,
                                        scalar2=-1.0, op0=ALU.mult, op1=ALU.mult)
                dt_ = dst3d[:, m, :]
                nc.vector.tensor_scalar(out=dt_, in0=xt, scalar1=ri,
                                        scalar2=nm, op0=ALU.mult, op1=ALU.add)
                nc.vector.tensor_mul(dt_, dt_, g_bc)
                nc.vector.tensor_add(dt_, dt_, b_bc)

        def transpose_local(xn3d, xnT3d, pt_pool):
            # [128, NTM, 1024] token-major -> [128, KD, NTM*128] feature-major
            for m in range(NTM):
                for k in range(KD):
                    pt = pt_pool.tile([128, 128], F32, tag="pt")
                    nc.tensor.transpose(
                        pt, xn3d[:, m, k * 128:(k + 1) * 128], identity)
                    nc.vector.tensor_copy(
                        out=xnT3d[:, k, m * 128:(m + 1) * 128], in_=pt)

        for l in range(L):
            # ---- LN1 local -> local transpose -> AllGather xn^T ----
            xn_my = xn_pool.tile([128, NTM, D], F32, tag="xn_my")
            layer_norm(x_my, ln1g.ap()[l:l + 1, :], ln1b.ap()[l:l + 1, :], xn_my)
            xnT_my = xnT_pool.tile([128, KD, TS], F32, tag="xnT_my")
            with tc.tile_pool(name=f"ptA{l}", bufs=4, space="PSUM") as ptA:
                transpose_local(xn_my, xnT_my, ptA)
            xnT_in = dram.tile([D, TS], F32, name=f"xnT_in{l}")
            xnT_out = dram.tile([NC * D, TS], F32, addr_space="Shared",
                                name=f"xnT_out{l}")
            nc.sync.dma_start(out=xnT_in.rearrange("(k p) t -> p k t", p=128),
                              in_=xnT_my)
            nc.gpsimd.collective_compute(
                "AllGather", ALU.bypass, replica_groups=[list(range(NC))],
                ins=[xnT_in[:]], outs=[xnT_out[:]])

            with tc.tile_pool(name=f"qk{l}", bufs=1) as qk_pool, \
                 tc.tile_pool(name=f"vtok{l}", bufs=1) as v_pool, \
                 tc.tile_pool(name=f"oT{l}", bufs=1) as oT_pool:
                qkT = qk_pool.tile([128, 2, TT], F32)    # q^T,k^T feat-major
                v_tok = v_pool.tile([128, TT // 128, 2 * HD], F32)
                oT = oT_pool.tile([128, TT], F32)

                # ---- QKV (2 local heads, all tokens) ----
                with tc.tile_pool(name=f"wq{l}", bufs=1) as wq_pool, \
                     tc.tile_pool(name=f"vT{l}", bufs=1) as vT_pool, \
                     tc.tile_pool(name=f"xc{l}", bufs=2) as xc_pool, \
                     tc.tile_pool(name=f"psA{l}", bufs=3, space="PSUM") as psA, \
                     tc.tile_pool(name=f"ptV{l}", bufs=4, space="PSUM") as ptV:
                    wq_sb = wq_pool.tile([128, KD, 3 * 128], F32)
                    nc.sync.dma_start(
                        out=wq_sb,
                        in_=wqkv.ap()[l].rearrange("(k p) m -> p k m", p=128))
                    vT = vT_pool.tile([128, TT], F32)
                    for r in range(NC):
                        xc = xc_pool.tile([128, KD, TS], F32, tag="xc")
                        nc.sync.dma_start(
                            out=xc,
                            in_=xnT_out[r * D:(r + 1) * D, :]
                            .rearrange("(k p) t -> p k t", p=128))
                        for m in range(3):
                            ps = psA.tile([128, TS], F32, tag="psA")
                            for k in range(KD):
                                nc.tensor.matmul(
                                    ps, wq_sb[:, k, m * 128:(m + 1) * 128],
                                    xc[:, k, :], start=(k == 0),
                                    stop=(k == KD - 1))
                            dst = (qkT[:, m, r * TS:(r + 1) * TS] if m < 2
                                   else vT[:, r * TS:(r + 1) * TS])
                            nc.vector.tensor_copy(out=dst, in_=ps)
                    # v -> token-major [128tok, 32, 128dims]
                    for ti in range(TT // 128):
                        pv = ptV.tile([128, 128], F32, tag="pv")
                        nc.tensor.transpose(
                            pv, vT[:, ti * 128:(ti + 1) * 128], identity)
                        nc.vector.tensor_copy(out=v_tok[:, ti, :], in_=pv)

                # ---- attention per (batch, local head, q-tile) ----
                with tc.tile_pool(name=f"att{l}", bufs=2) as att_pool, \
                     tc.tile_pool(name=f"pTs{l}", bufs=8) as pT_pool, \
                     tc.tile_pool(name=f"asc{l}", bufs=4) as att_sc, \
                     tc.tile_pool(name=f"psS{l}", bufs=2, space="PSUM") as psS, \
                     tc.tile_pool(name=f"ptT{l}", bufs=2, space="PSUM") as ptT, \
                     tc.tile_pool(name=f"psO{l}", bufs=2, space="PSUM") as psO:
                    for b in range(B):
                        for h in range(HL):
                            hp = h * HD
                            for qi in range(NQ):
                                nk = (qi + 1) * 128
                                qcol = b * T + qi * 128
                                ps_s = psS.tile([128, 1024], F32, tag="ps_s")
                                for jn in range((nk + 511) // 512):
                                    j0 = jn * 512
                                    jw = min(512, nk - j0)
                                    nc.tensor.matmul(
                                        ps_s[:, j0:j0 + jw],
                                        qkT[hp:hp + HD, 0, qcol:qcol + 128],
                                        qkT[hp:hp + HD, 1,
                                            b * T + j0:b * T + j0 + jw],
                                        start=True, stop=True)
                                nc.vector.tensor_add(
                                    ps_s[:, nk - 128:nk], ps_s[:, nk - 128:nk],
                                    mask128)
                                mx = att_sc.tile([128, 1], F32, tag="mx")
                                nc.vector.reduce_max(out=mx, in_=ps_s[:, :nk],
                                                     axis=AX)
                                be = att_sc.tile([128, 1], F32, tag="be")
                                nc.vector.tensor_scalar(
                                    out=be, in0=mx, scalar1=-SCALE,
                                    scalar2=None, op0=ALU.mult)
                                p_sb = att_pool.tile([128, 1024], F32,
                                                     tag="p_sb")
                                rs = att_sc.tile([128, 1], F32, tag="rs")
                                nc.scalar.activation(
                                    out=p_sb[:, :nk], in_=ps_s[:, :nk],
                                    func=ACTF.Exp, bias=be, scale=SCALE,
                                    accum_out=rs)
                                ri = att_sc.tile([128, 1], F32, tag="ri2")
                                nc.vector.reciprocal(out=ri, in_=rs)
                                nc.vector.tensor_scalar_mul(
                                    p_sb[:, :nk], p_sb[:, :nk], ri)
                                pTs = []
                                for jk in range(qi + 1):
                                    ptp = ptT.tile([128, 128], F32, tag="ptp")
                                    nc.tensor.transpose(
                                        ptp, p_sb[:, jk * 128:(jk + 1) * 128],
                                        identity)
                                    pT = pT_pool.tile([128, 128], F32,
                                                      tag="pT")
                                    nc.vector.tensor_copy(out=pT, in_=ptp)
                                    pTs.append(pT)
                                ps_o = psO.tile([64, 128], F32, tag="ps_o")
                                for jk in range(qi + 1):
                                    nc.tensor.matmul(
                                        ps_o, v_tok[:, b * NQ + jk,
                                                    hp:hp + HD],
                                        pTs[jk], start=(jk == 0),
                                        stop=(jk == qi))
                                nc.scalar.activation(
                                    out=oT[hp:hp + HD, qcol:qcol + 128],
                                    in_=ps_o, func=ACTF.Copy)

                # ---- AllToAll o^T: receive oT_full[:, my tokens] ----
                oTa_in = dram.tile([NC * 128, TS], F32, name=f"oTa_in{l}")
                oTa_out = dram.tile([NC * 128, TS], F32, name=f"oTa_out{l}")
                nc.sync.dma_start(
                    out=oTa_in.rearrange("(j p) t -> p j t", p=128),
                    in_=oT.rearrange("p (j t) -> p j t", j=NC))
                nc.gpsimd.collective_compute(
                    "AllToAll", ALU.bypass, replica_groups=[list(range(NC))],
                    ins=[oTa_in[:]], outs=[oTa_out[:]])

            # ---- proj on my 512 tokens + residual ----
            with tc.tile_pool(name=f"om{l}", bufs=1) as om_pool, \
                 tc.tile_pool(name=f"wp{l}", bufs=2) as wp_pool, \
                 tc.tile_pool(name=f"psP{l}", bufs=3, space="PSUM") as psP:
                bp_bc = bc_pool.tile([128, D], F32, tag="g_bc")
                nc.sync.dma_start(out=bp_bc, in_=_bcast(bproj.ap()[l:l + 1, :]))
                oTmy = om_pool.tile([128, KD, TS], F32)
                nc.sync.dma_start(
                    out=oTmy,
                    in_=oTa_out[:].rearrange("(k p) t -> p k t", p=128))
                for n in range(2):
                    wp_sb = wp_pool.tile([128, KD, 512], F32, tag="wp")
                    nc.sync.dma_start(
                        out=wp_sb,
                        in_=wproj.ap()[l][:, n * 512:(n + 1) * 512]
                        .rearrange("(k p) d -> p k d", p=128))
                    for m in range(NTM):
                        ps = psP.tile([128, 512], F32, tag="psP")
                        for k in range(KD):
                            nc.tensor.matmul(
                                ps, oTmy[:, k, m * 128:(m + 1) * 128],
                                wp_sb[:, k, :], start=(k == 0),
                                stop=(k == KD - 1))
                        xs = x_my[:, m, n * 512:(n + 1) * 512]
                        nc.vector.tensor_add(xs, xs, ps)
                        nc.vector.tensor_add(
                            xs, xs, bp_bc[:, n * 512:(n + 1) * 512])

            # ---- LN2 local -> transpose -> FFN -> residual ----
            xn2 = xn_pool.tile([128, NTM, D], F32, tag="xn_my")
            layer_norm(x_my, ln2g.ap()[l:l + 1, :], ln2b.ap()[l:l + 1, :], xn2)
            xn2T = xnT_pool.tile([128, KD, TS], F32, tag="xnT_my")
            with tc.tile_pool(name=f"ptB{l}", bufs=4, space="PSUM") as ptB:
                transpose_local(xn2, xn2T, ptB)

            with tc.tile_pool(name=f"hT{l}", bufs=1) as hT_pool, \
                 tc.tile_pool(name=f"w1s{l}", bufs=3) as w1_pool, \
                 tc.tile_pool(name=f"b1s{l}", bufs=1) as b1_pool:
                hT = hT_pool.tile([128, KF, TS], F32)
                b1_sb = b1_pool.tile([128, KF], F32)
                nc.sync.dma_start(
                    out=b1_sb, in_=b1.ap()[l].rearrange("(m p) -> p m", p=128))
                with tc.tile_pool(name=f"psF{l}", bufs=4, space="PSUM") as psF:
                    for mh in range(KF):
                        w1_sb = w1_pool.tile([128, KD, 128], F32, tag="w1")
                        nc.sync.dma_start(
                            out=w1_sb,
                            in_=w1.ap()[l][:, mh * 128:(mh + 1) * 128]
                            .rearrange("(k p) m -> p k m", p=128))
                        ps = psF.tile([128, TS], F32, tag="psF")
                        for k in range(KD):
                            nc.tensor.matmul(ps, w1_sb[:, k, :], xn2T[:, k, :],
                                             start=(k == 0), stop=(k == KD - 1))
                        nc.scalar.activation(
                            out=hT[:, mh, :], in_=ps, func=ACTF.Relu,
                            bias=b1_sb[:, mh:mh + 1], scale=1.0)

                b2_bc = bc_pool.tile([128, D], F32, tag="b_bc")
                nc.sync.dma_start(out=b2_bc, in_=_bcast(b2.ap()[l:l + 1, :]))
                with tc.tile_pool(name=f"w2s{l}", bufs=3) as w2_pool, \
                     tc.tile_pool(name=f"psG{l}", bufs=1, space="PSUM") as psG:
                    for n in range(2):
                        ps_m = [psG.tile([128, 512], F32, tag=f"psG{m}",
                                         name=f"psG{l}_{n}_{m}")
                                for m in range(NTM)]
                        for kg in range(4):
                            w2_sb = w2_pool.tile([128, 8, 512], F32, tag="w2")
                            nc.sync.dma_start(
                                out=w2_sb,
                                in_=w2.ap()[l][kg * 1024:(kg + 1) * 1024,
                                               n * 512:(n + 1) * 512]
                                .rearrange("(k p) d -> p k d", p=128))
                            for m in range(NTM):
                                for k8 in range(8):
                                    kh = kg * 8 + k8
                                    nc.tensor.matmul(
                                        ps_m[m],
                                        hT[:, kh, m * 128:(m + 1) * 128],
                                        w2_sb[:, k8, :],
                                        start=(kh == 0), stop=(kh == KF - 1))
                        for m in range(NTM):
                            xs = x_my[:, m, n * 512:(n + 1) * 512]
                            nc.vector.tensor_add(xs, xs, ps_m[m])
                            nc.vector.tensor_add(
                                xs, xs, b2_bc[:, n * 512:(n + 1) * 512])

        # ---- final LN -> transpose -> AllGather xf^T ----
        xf = xn_pool.tile([128, NTM, D], F32, tag="xn_my")
        layer_norm(x_my, lnfg.ap()[0:1, :], lnfb.ap()[0:1, :], xf)
        xfT_my = xnT_pool.tile([128, KD, TS], F32, tag="xnT_my")
        with tc.tile_pool(name="ptF", bufs=4, space="PSUM") as ptF:
            transpose_local(xf, xfT_my, ptF)
        xfT_in = dram.tile([D, TS], F32, name="xfT_in")
        xfT_out = dram.tile([NC * D, TS], F32, addr_space="Shared",
                            name="xfT_out")
        nc.sync.dma_start(out=xfT_in.rearrange("(k p) t -> p k t", p=128),
                          in_=xfT_my)
        nc.gpsimd.collective_compute(
            "AllGather", ALU.bypass, replica_groups=[list(range(NC))],
            ins=[xfT_in[:]], outs=[xfT_out[:]])

        # ---- LM head: all 4096 tokens x my 4000 vocab ----
        nvc = (VC + 511) // 512  # 8 chunks (last = 416)
        with tc.tile_pool(name="wlm", bufs=2) as wlm_pool, \
             tc.tile_pool(name="blml", bufs=1) as blm_pool, \
             tc.tile_pool(name="xfTl", bufs=3) as xfT_pool, \
             tc.tile_pool(name="lg", bufs=4) as lg_pool, \
             tc.tile_pool(name="psL", bufs=4, space="PSUM") as psL:
            blm_bc = blm_pool.tile([128, VC], F32)
            nc.sync.dma_start(out=blm_bc, in_=_bcast(blm.ap()[0:1, :]))
            for nv in range(nvc):
                n0 = nv * 512
                nw = min(512, VC - n0)
                wlm_sb = wlm_pool.tile([128, KD, 512], F32, tag="wlm")
                nc.sync.dma_start(
                    out=wlm_sb[:, :, :nw],
                    in_=wlm.ap()[:, n0:n0 + nw]
                    .rearrange("(k p) v -> p k v", p=128))
                for mt in range(TT // 128):
                    r, tl = mt // NTM, mt % NTM
                    xfT_sb = xfT_pool.tile([128, KD, 128], F32, tag="xfT")
                    nc.sync.dma_start(
                        out=xfT_sb,
                        in_=xfT_out[r * D:(r + 1) * D, tl * 128:(tl + 1) * 128]
                        .rearrange("(k p) t -> p k t", p=128))
                    ps = psL.tile([128, 512], F32, tag="psL")
                    for k in range(KD):
                        nc.tensor.matmul(
                            ps[:, :nw], xfT_sb[:, k, :],
                            wlm_sb[:, k, :nw],
                            start=(k == 0), stop=(k == KD - 1))
                    lg = lg_pool.tile([128, 512], F32, tag="lg")
                    nc.vector.tensor_add(lg[:, :nw], ps[:, :nw],
                                         blm_bc[:, n0:n0 + nw])
                    nc.sync.dma_start(
                        out=logits.ap()[mt * 128:(mt + 1) * 128, n0:n0 + nw],
                        in_=lg[:, :nw])


def _prep_inputs(inputs):
    f = lambda a: np.ascontiguousarray(np.asarray(a), dtype=np.float32)
    idx = np.asarray(inputs["idx"])
    tok_emb, pos_emb = f(inputs["tok_emb"]), f(inputs["pos_emb"])
    xfull = (tok_emb[idx.reshape(-1)]
             + np.tile(pos_emb[:T], (B, 1))).astype(np.float32)  # [4096, D]
    wq, wk, wv = f(inputs["wq"]), f(inputs["wk"]), f(inputs["wv"])
    wlm_f, blm_f = f(inputs["wlm"]), f(inputs["blm"])

    shared = dict(
        wproj=f(inputs["wproj"]), bproj=f(inputs["bproj"]),
        ln1g=f(inputs["ln1_g"]), ln1b=f(inputs["ln1_b"]),
        ln2g=f(inputs["ln2_g"]), ln2b=f(inputs["ln2_b"]),
        w1=f(inputs["w1"]), b1=f(inputs["b1"]),
        w2=f(inputs["w2"]), b2=f(inputs["b2"]),
        lnfg=f(inputs["lnf_g"]).reshape(1, D),
        lnfb=f(inputs["lnf_b"]).reshape(1, D),
    )
    in_maps = []
    for c in range(NC):
        h0 = HL * c
        wqkv_c = np.concatenate(
            [w[:, h0:h0 + HL].transpose(0, 2, 1, 3).reshape(L, D, HL * HD)
             for w in (wq, wk, wv)], axis=2)  # [L, D, 384]
        m = dict(shared)
        m["x0"] = np.ascontiguousarray(xfull[c * TS:(c + 1) * TS])
        m["wqkv"] = np.ascontiguousarray(wqkv_c)
        m["wlm"] = np.ascontiguousarray(wlm_f[:, c * VC:(c + 1) * VC])
        m["blm"] = np.ascontiguousarray(blm_f[c * VC:(c + 1) * VC].reshape(1, VC))
        in_maps.append(m)
    return in_maps


def kernel(**inputs):
    nc = build_program()
    in_maps = _prep_inputs(inputs)
    res = run_bass_kernel_spmd(nc, in_maps, core_ids=list(range(NC)))
    logits = np.concatenate([res.results[c]["logits"] for c in range(NC)],
                            axis=1)  # [4096, 32000]
    target = np.asarray(inputs["target"]).reshape(-1)
    mx = logits.max(axis=-1)
    lse = mx + np.log(np.exp(logits - mx[:, None]).sum(axis=-1))
    tl = logits[np.arange(TT), target]
    loss = np.float32(np.mean(lse - tl))
    return logits.reshape(B, T, V), loss


# revision 9
# speedup vs baseline: 2.7626x; 2.7626x over previous
"""GPT forward (L=3, D=1024, H=16, T=1024, B=4, V=32000) on 8 trn2 cores.

Sharding: attention head-parallel (2 heads/core); proj+FFN+LN
token-parallel (512 tokens/core); LM head vocab-parallel (4000/core).
Collectives per layer: AllGather(xn^T, 2.1MB) before QKV and
AllToAll(o^T, 2.1MB) before the proj (A2A hands each core exactly
oT_full[:, my_tokens] at a static address). One final AllGather(xf^T)
before the LM head. Activations are kept feature-major for GEMMs via
128x128 PE transposes; the residual stream stays token-major per core.
Embedding lookup and the loss reduction run on host.
"""
import sys

for _p in ("/opt/trn_rl_repo",):
    if _p not in sys.path:
        sys.path.insert(0, _p)

import numpy as np

import concourse.bass as bass
import concourse.mybir as mybir
import concourse.tile as tile
from concourse import bacc
from concourse.bass_utils import run_bass_kernel_spmd
from concourse.masks import make_causal_mask, make_identity

F32 = mybir.dt.float32
BF16 = mybir.dt.bfloat16
AX = mybir.AxisListType.X
ALU = mybir.AluOpType
ACTF = mybir.ActivationFunctionType

NC = 8          # cores
V, BLK, H, D, L = 32000, 1024, 16, 1024, 3
HD = D // H     # 64
B, T = 4, 1024
TT = B * T      # 4096 tokens total
TS = TT // NC   # 512 tokens per core
NTM = TS // 128  # 4 token tiles per core
KD = D // 128    # 8 feature chunks
HL = H // NC     # 2 local heads
FF = 4 * D       # 4096
KF = FF // 128   # 32 hidden chunks
VC = V // NC     # 4000 vocab per core
NQ = T // 128    # 8 q tiles per batch
SCALE = float(D) ** -0.5
MASKVAL = -1e9
EPS = 1e-5

_CACHE = {}


def _bcast(ap_row, parts=128):
    """row AP [1, n] -> [parts, n] partition-broadcast (stride-0) AP."""
    return bass.AP(tensor=ap_row.tensor, offset=ap_row.offset,
                   ap=[[0, parts]] + [list(ap_row.ap[-1])])


def build_program():
    if "nc" in _CACHE:
        return _CACHE["nc"]
    nc = bacc.Bacc("TRN2", target_bir_lowering=False, debug=False,
                   enable_asserts=True, num_devices=NC)

    x0 = nc.dram_tensor("x0", [TS, D], F32, kind="ExternalInput")
    wqkv = nc.dram_tensor("wqkv", [L, D, 3 * 2 * HD], BF16, kind="ExternalInput")
    wproj = nc.dram_tensor("wproj", [L, D, D], BF16, kind="ExternalInput")
    bproj = nc.dram_tensor("bproj", [L, D], F32, kind="ExternalInput")
    ln1g = nc.dram_tensor("ln1g", [L, D], F32, kind="ExternalInput")
    ln1b = nc.dram_tensor("ln1b", [L, D], F32, kind="ExternalInput")
    ln2g = nc.dram_tensor("ln2g", [L, D], F32, kind="ExternalInput")
    ln2b = nc.dram_tensor("ln2b", [L, D], F32, kind="ExternalInput")
    w1 = nc.dram_tensor("w1", [L, D, FF], BF16, kind="ExternalInput")
    b1 = nc.dram_tensor("b1", [L, FF], F32, kind="ExternalInput")
    w2 = nc.dram_tensor("w2", [L, FF, D], BF16, kind="ExternalInput")
    b2 = nc.dram_tensor("b2", [L, D], F32, kind="ExternalInput")
    lnfg = nc.dram_tensor("lnfg", [1, D], F32, kind="ExternalInput")
    lnfb = nc.dram_tensor("lnfb", [1, D], F32, kind="ExternalInput")
    wlm = nc.dram_tensor("wlm", [D, VC], BF16, kind="ExternalInput")
    blm = nc.dram_tensor("blm", [1, VC], F32, kind="ExternalInput")
    logits = nc.dram_tensor("logits", [TT, VC], F32, kind="ExternalOutput")

    with tile.TileContext(nc) as tc:
        _emit(nc, tc, x0, wqkv, wproj, bproj, ln1g, ln1b, ln2g, ln2b,
              w1, b1, w2, b2, lnfg, lnfb, wlm, blm, logits)
    nc.compile()
    _CACHE["nc"] = nc
    return nc


def _emit(nc, tc, x0, wqkv, wproj, bproj, ln1g, ln1b, ln2g, ln2b,
          w1, b1, w2, b2, lnfg, lnfb, wlm, blm, logits):
    from contextlib import ExitStack
    with ExitStack() as ctx:
        const = ctx.enter_context(tc.tile_pool(name="const", bufs=1))
        identity = const.tile([128, 128], BF16)
        make_identity(nc, identity)
        mask128 = const.tile([128, 128], F32)
        make_causal_mask(nc, mask128, mask_val=MASKVAL)
        eps_t = const.tile([128, 1], F32)
        nc.vector.memset(eps_t, EPS)

        persist = ctx.enter_context(tc.tile_pool(name="persist", bufs=1))
        x_my = persist.tile([128, NTM, D], F32)  # residual, token-major
        nc.sync.dma_start(out=x_my, in_=x0.ap().rearrange("(t p) d -> p t d", p=128))

        dram = ctx.enter_context(tc.tile_pool(name="dram", bufs=1, space="DRAM"))
        ln_sc = ctx.enter_context(tc.tile_pool(name="ln_sc", bufs=4))
        bc_pool = ctx.enter_context(tc.tile_pool(name="bc", bufs=2))
        xn_pool = ctx.enter_context(tc.tile_pool(name="xn", bufs=1))
        xnT_pool = ctx.enter_context(tc.tile_pool(name="xnT", bufs=1))

        def layer_norm(src3d, g_row, b_row, dst3d):
            g_bc = bc_pool.tile([128, D], F32, tag="g_bc")
            b_bc = bc_pool.tile([128, D], F32, tag="b_bc")
            nc.sync.dma_start(out=g_bc, in_=_bcast(g_row))
            nc.sync.dma_start(out=b_bc, in_=_bcast(b_row))
            for m in range(NTM):
                xt = src3d[:, m, :]
                stats = ln_sc.tile([128, 2, 6], F32, tag="stats")
                nc.vector.bn_stats(out=stats[:, 0, :], in_=xt[:, 0:512])
                nc.vector.bn_stats(out=stats[:, 1, :], in_=xt[:, 512:1024])
                mv = ln_sc.tile([128, 2], F32, tag="mv")
                nc.vector.bn_aggr(out=mv, in_=stats)
                sd = ln_sc.tile([128, 1], F32, tag="sd")
                nc.scalar.activation(out=sd, in_=mv[:, 1:2], func=ACTF.Sqrt,
                                     bias=eps_t, scale=1.0)
                ri = ln_sc.tile([128, 1], F32, tag="ri")
                nc.vector.reciprocal(out=ri, in_=sd)
                nm = ln_sc.tile([128, 1], F32, tag="nm")
                nc.vector.tensor_scalar(out=nm, in0=mv[:, 0:1], scalar1=ri,
                                        scalar2=-1.0, op0=ALU.mult, op1=ALU.mult)
                tmp = ln_sc.tile([128, D], F32, tag="tmp")
                nc.vector.tensor_scalar(out=tmp, in0=xt, scalar1=ri,
                                        scalar2=nm, op0=ALU.mult, op1=ALU.add)
                dt_ = dst3d[:, m, :]
                nc.vector.tensor_mul(dt_, tmp, g_bc)
                nc.vector.tensor_add(dt_, dt_, b_bc)

        def transpose_local(xn3d, xnT3d, pt_pool):
            # [128, NTM, 1024] token-major -> [128, KD, NTM*128] feature-major
            for m in range(NTM):
                for k in range(KD):
                    pt = pt_pool.tile([128, 128], BF16, tag="pt")
                    nc.tensor.transpose(
                        pt, xn3d[:, m, k * 128:(k + 1) * 128], identity)
                    nc.vector.tensor_copy(
                        out=xnT3d[:, k, m * 128:(m + 1) * 128], in_=pt)

        for l in range(L):
            # ---- LN1 local -> local transpose -> AllGather xn^T ----
            xn_my = xn_pool.tile([128, NTM, D], BF16, tag="xn_my")
            layer_norm(x_my, ln1g.ap()[l:l + 1, :], ln1b.ap()[l:l + 1, :], xn_my)
            xnT_my = xnT_pool.tile([128, KD, TS], BF16, tag="xnT_my")
            with tc.tile_pool(name=f"ptA{l}", bufs=4, space="PSUM") as ptA:
                transpose_local(xn_my, xnT_my, ptA)
            xnT_in = dram.tile([D, TS], BF16, name=f"xnT_in{l}")
            xnT_out = dram.tile([NC * D, TS], BF16, addr_space="Shared",
                                name=f"xnT_out{l}")
            nc.sync.dma_start(out=xnT_in.rearrange("(k p) t -> p k t", p=128),
                              in_=xnT_my)
            nc.gpsimd.collective_compute(
                "AllGather", ALU.bypass, replica_groups=[list(range(NC))],
                ins=[xnT_in[:]], outs=[xnT_out[:]])

            with tc.tile_pool(name=f"qk{l}", bufs=1) as qk_pool, \
                 tc.tile_pool(name=f"vtok{l}", bufs=1) as v_pool, \
                 tc.tile_pool(name=f"oT{l}", bufs=1) as oT_pool:
                qkT = qk_pool.tile([128, 2, TT], BF16)    # q^T,k^T feat-major
                v_tok = v_pool.tile([128, TT // 128, 2 * HD], BF16)
                oT = oT_pool.tile([128, TT], BF16)

                # ---- QKV (2 local heads, all tokens) ----
                with tc.tile_pool(name=f"wq{l}", bufs=1) as wq_pool, \
                     tc.tile_pool(name=f"vT{l}", bufs=1) as vT_pool, \
                     tc.tile_pool(name=f"xc{l}", bufs=2) as xc_pool, \
                     tc.tile_pool(name=f"psA{l}", bufs=3, space="PSUM") as psA, \
                     tc.tile_pool(name=f"ptV{l}", bufs=4, space="PSUM") as ptV:
                    wq_sb = wq_pool.tile([128, KD, 3 * 128], BF16)
                    nc.sync.dma_start(
                        out=wq_sb,
                        in_=wqkv.ap()[l].rearrange("(k p) m -> p k m", p=128))
                    vT = vT_pool.tile([128, TT], BF16)
                    for r in range(NC):
                        xc = xc_pool.tile([128, KD, TS], BF16, tag="xc")
                        nc.sync.dma_start(
                            out=xc,
                            in_=xnT_out[r * D:(r + 1) * D, :]
                            .rearrange("(k p) t -> p k t", p=128))
                        for m in range(3):
                            ps = psA.tile([128, TS], F32, tag="psA")
                            for k in range(KD):
                                nc.tensor.matmul(
                                    ps, wq_sb[:, k, m * 128:(m + 1) * 128],
                                    xc[:, k, :], start=(k == 0),
                                    stop=(k == KD - 1))
                            dst = (qkT[:, m, r * TS:(r + 1) * TS] if m < 2
                                   else vT[:, r * TS:(r + 1) * TS])
                            nc.vector.tensor_copy(out=dst, in_=ps)
                    # v -> token-major [128tok, 32, 128dims]
                    for ti in range(TT // 128):
                        pv = ptV.tile([128, 128], BF16, tag="pv")
                        nc.tensor.transpose(
                            pv, vT[:, ti * 128:(ti + 1) * 128], identity)
                        nc.vector.tensor_copy(out=v_tok[:, ti, :], in_=pv)

                # ---- attention per (batch, local head, q-tile) ----
                with tc.tile_pool(name=f"att{l}", bufs=2) as att_pool, \
                     tc.tile_pool(name=f"pTs{l}", bufs=8) as pT_pool, \
                     tc.tile_pool(name=f"asc{l}", bufs=4) as att_sc, \
                     tc.tile_pool(name=f"psS{l}", bufs=2, space="PSUM") as psS, \
                     tc.tile_pool(name=f"ptT{l}", bufs=2, space="PSUM") as ptT, \
                     tc.tile_pool(name=f"psO{l}", bufs=2, space="PSUM") as psO:
                    for b in range(B):
                        for h in range(HL):
                            hp = h * HD
                            for qi in range(NQ):
                                nk = (qi + 1) * 128
                                qcol = b * T + qi * 128
                                ps_s = psS.tile([128, 1024], F32, tag="ps_s")
                                for jn in range((nk + 511) // 512):
                                    j0 = jn * 512
                                    jw = min(512, nk - j0)
                                    nc.tensor.matmul(
                                        ps_s[:, j0:j0 + jw],
                                        qkT[hp:hp + HD, 0, qcol:qcol + 128],
                                        qkT[hp:hp + HD, 1,
                                            b * T + j0:b * T + j0 + jw],
                                        start=True, stop=True)
                                nc.vector.tensor_add(
                                    ps_s[:, nk - 128:nk], ps_s[:, nk - 128:nk],
                                    mask128)
                                mx = att_sc.tile([128, 1], F32, tag="mx")
                                nc.vector.reduce_max(out=mx, in_=ps_s[:, :nk],
                                                     axis=AX)
                                be = att_sc.tile([128, 1], F32, tag="be")
                                nc.vector.tensor_scalar(
                                    out=be, in0=mx, scalar1=-SCALE,
                                    scalar2=None, op0=ALU.mult)
                                p_sb = att_pool.tile([128, 1024], BF16,
                                                     tag="p_sb")
                                rs = att_sc.tile([128, 1], F32, tag="rs")
                                nc.scalar.activation(
                                    out=p_sb[:, :nk], in_=ps_s[:, :nk],
                                    func=ACTF.Exp, bias=be, scale=SCALE,
                                    accum_out=rs)
                                ri = att_sc.tile([128, 1], F32, tag="ri2")
                                nc.vector.reciprocal(out=ri, in_=rs)
                                nc.vector.tensor_scalar_mul(
                                    p_sb[:, :nk], p_sb[:, :nk], ri)
                                pTs = []
                                for jk in range(qi + 1):
                                    ptp = ptT.tile([128, 128], BF16, tag="ptp")
                                    nc.tensor.transpose(
                                        ptp, p_sb[:, jk * 128:(jk + 1) * 128],
                                        identity)
                                    pT = pT_pool.tile([128, 128], BF16,
                                                      tag="pT")
                                    nc.vector.tensor_copy(out=pT, in_=ptp)
                                    pTs.append(pT)
                                ps_o = psO.tile([64, 128], F32, tag="ps_o")
                                for jk in range(qi + 1):
                                    nc.tensor.matmul(
                                        ps_o, v_tok[:, b * NQ + jk,
                                                    hp:hp + HD],
                                        pTs[jk], start=(jk == 0),
                                        stop=(jk == qi))
                                nc.scalar.activation(
                                    out=oT[hp:hp + HD, qcol:qcol + 128],
                                    in_=ps_o, func=ACTF.Copy)

                # ---- AllToAll o^T: receive oT_full[:, my tokens] ----
                oTa_in = dram.tile([NC * 128, TS], BF16, name=f"oTa_in{l}")
                oTa_out = dram.tile([NC * 128, TS], BF16, name=f"oTa_out{l}")
                nc.sync.dma_start(
                    out=oTa_in.rearrange("(j p) t -> p j t", p=128),
                    in_=oT.rearrange("p (j t) -> p j t", j=NC))
                nc.gpsimd.collective_compute(
                    "AllToAll", ALU.bypass, replica_groups=[list(range(NC))],
                    ins=[oTa_in[:]], outs=[oTa_out[:]])

            # ---- proj on my 512 tokens + residual ----
            with tc.tile_pool(name=f"om{l}", bufs=1) as om_pool, \
                 tc.tile_pool(name=f"wp{l}", bufs=2) as wp_pool, \
                 tc.tile_pool(name=f"psP{l}", bufs=3, space="PSUM") as psP:
                bp_bc = bc_pool.tile([128, D], F32, tag="g_bc")
                nc.sync.dma_start(out=bp_bc, in_=_bcast(bproj.ap()[l:l + 1, :]))
                oTmy = om_pool.tile([128, KD, TS], BF16)
                nc.sync.dma_start(
                    out=oTmy,
                    in_=oTa_out[:].rearrange("(k p) t -> p k t", p=128))
                for n in range(2):
                    wp_sb = wp_pool.tile([128, KD, 512], BF16, tag="wp")
                    nc.sync.dma_start(
                        out=wp_sb,
                        in_=wproj.ap()[l][:, n * 512:(n + 1) * 512]
                        .rearrange("(k p) d -> p k d", p=128))
                    for m in range(NTM):
                        ps = psP.tile([128, 512], F32, tag="psP")
                        for k in range(KD):
                            nc.tensor.matmul(
                                ps, oTmy[:, k, m * 128:(m + 1) * 128],
                                wp_sb[:, k, :], start=(k == 0),
                                stop=(k == KD - 1))
                        xs = x_my[:, m, n * 512:(n + 1) * 512]
                        nc.vector.tensor_add(xs, xs, ps)
                        nc.vector.tensor_add(
                            xs, xs, bp_bc[:, n * 512:(n + 1) * 512])

            # ---- LN2 local -> transpose -> FFN -> residual ----
            xn2 = xn_pool.tile([128, NTM, D], BF16, tag="xn_my")
            layer_norm(x_my, ln2g.ap()[l:l + 1, :], ln2b.ap()[l:l + 1, :], xn2)
            xn2T = xnT_pool.tile([128, KD, TS], BF16, tag="xnT_my")
            with tc.tile_pool(name=f"ptB{l}", bufs=4, space="PSUM") as ptB:
                transpose_local(xn2, xn2T, ptB)

            with tc.tile_pool(name=f"hT{l}", bufs=1) as hT_pool, \
                 tc.tile_pool(name=f"w1s{l}", bufs=3) as w1_pool, \
                 tc.tile_pool(name=f"b1s{l}", bufs=1) as b1_pool:
                hT = hT_pool.tile([128, KF, TS], BF16)
                b1_sb = b1_pool.tile([128, KF], F32)
                nc.sync.dma_start(
                    out=b1_sb, in_=b1.ap()[l].rearrange("(m p) -> p m", p=128))
                with tc.tile_pool(name=f"psF{l}", bufs=4, space="PSUM") as psF:
                    for mh in range(KF):
                        w1_sb = w1_pool.tile([128, KD, 128], BF16, tag="w1")
                        nc.sync.dma_start(
                            out=w1_sb,
                            in_=w1.ap()[l][:, mh * 128:(mh + 1) * 128]
                            .rearrange("(k p) m -> p k m", p=128))
                        ps = psF.tile([128, TS], F32, tag="psF")
                        for k in range(KD):
                            nc.tensor.matmul(ps, w1_sb[:, k, :], xn2T[:, k, :],
                                             start=(k == 0), stop=(k == KD - 1))
                        nc.scalar.activation(
                            out=hT[:, mh, :], in_=ps, func=ACTF.Relu,
                            bias=b1_sb[:, mh:mh + 1], scale=1.0)

                b2_bc = bc_pool.tile([128, D], F32, tag="b_bc")
                nc.sync.dma_start(out=b2_bc, in_=_bcast(b2.ap()[l:l + 1, :]))
                with tc.tile_pool(name=f"w2s{l}", bufs=3) as w2_pool, \
                     tc.tile_pool(name=f"psG{l}", bufs=1, space="PSUM") as psG:
                    for n in range(2):
                        ps_m = [psG.tile([128, 512], F32, tag=f"psG{m}",
                                         name=f"psG{l}_{n}_{m}")
                                for m in range(NTM)]
                        for kg in range(4):
                            w2_sb = w2_pool.tile([128, 8, 512], BF16, tag="w2")
                            nc.sync.dma_start(
                                out=w2_sb,
                                in_=w2.ap()[l][kg * 1024:(kg + 1) * 1024,
                                               n * 512:(n + 1) * 512]
                                .rearrange("(k p) d -> p k d", p=128))
                            for m in range(NTM):
                                for k8 in range(8):
                                    kh = kg * 8 + k8
                                    nc.tensor.matmul(
                                        ps_m[m],
                                        hT[:, kh, m * 128:(m + 1) * 128],
                                        w2_sb[:, k8, :],
                                        start=(kh == 0), stop=(kh == KF - 1))
                        for m in range(NTM):
                            xs = x_my[:, m, n * 512:(n + 1) * 512]
                            nc.vector.tensor_add(xs, xs, ps_m[m])
                            nc.vector.tensor_add(
                                xs, xs, b2_bc[:, n * 512:(n + 1) * 512])

        # ---- final LN -> transpose -> AllGather xf^T ----
        xf = xn_pool.tile([128, NTM, D], BF16, tag="xn_my")
        layer_norm(x_my, lnfg.ap()[0:1, :], lnfb.ap()[0:1, :], xf)
        xfT_my = xnT_pool.tile([128, KD, TS], BF16, tag="xnT_my")
        with tc.tile_pool(name="ptF", bufs=4, space="PSUM") as ptF:
            transpose_local(xf, xfT_my, ptF)
        xfT_in = dram.tile([D, TS], BF16, name="xfT_in")
        xfT_out = dram.tile([NC * D, TS], BF16, addr_space="Shared",
                            name="xfT_out")
        nc.sync.dma_start(out=xfT_in.rearrange("(k p) t -> p k t", p=128),
                          in_=xfT_my)
        nc.gpsimd.collective_compute(
            "AllGather", ALU.bypass, replica_groups=[list(range(NC))],
            ins=[xfT_in[:]], outs=[xfT_out[:]])

        # ---- LM head: all 4096 tokens x my 4000 vocab ----
        nvc = (VC + 511) // 512  # 8 chunks (last = 416)
        with tc.tile_pool(name="wlm", bufs=2) as wlm_pool, \
             tc.tile_pool(name="blml", bufs=1) as blm_pool, \
             tc.tile_pool(name="xfTl", bufs=3) as xfT_pool, \
             tc.tile_pool(name="lg", bufs=4) as lg_pool, \
             tc.tile_pool(name="psL", bufs=4, space="PSUM") as psL:
            blm_bc = blm_pool.tile([128, VC], F32)
            nc.sync.dma_start(out=blm_bc, in_=_bcast(blm.ap()[0:1, :]))
            for nv in range(nvc):
                n0 = nv * 512
                nw = min(512, VC - n0)
                wlm_sb = wlm_pool.tile([128, KD, 512], BF16, tag="wlm")
                nc.sync.dma_start(
                    out=wlm_sb[:, :, :nw],
                    in_=wlm.ap()[:, n0:n0 + nw]
                    .rearrange("(k p) v -> p k v", p=128))
                for mt in range(TT // 128):
                    r, tl = mt // NTM, mt % NTM
                    xfT_sb = xfT_pool.tile([128, KD, 128], BF16, tag="xfT")
                    nc.sync.dma_start(
                        out=xfT_sb,
                        in_=xfT_out[r * D:(r + 1) * D, tl * 128:(tl + 1) * 128]
                        .rearrange("(k p) t -> p k t", p=128))
                    ps = psL.tile([128, 512], F32, tag="psL")
                    for k in range(KD):
                        nc.tensor.matmul(
                            ps[:, :nw], xfT_sb[:, k, :],
                            wlm_sb[:, k, :nw],
                            start=(k == 0), stop=(k == KD - 1))
                    lg = lg_pool.tile([128, 512], F32, tag="lg")
                    nc.vector.tensor_add(lg[:, :nw], ps[:, :nw],
                                         blm_bc[:, n0:n0 + nw])
                    nc.sync.dma_start(
                        out=logits.ap()[mt * 128:(mt + 1) * 128, n0:n0 + nw],
                        in_=lg[:, :nw])


def _prep_inputs(inputs):
    import ml_dtypes
    BF = ml_dtypes.bfloat16
    f = lambda a: np.ascontiguousarray(np.asarray(a), dtype=np.float32)
    idx = np.asarray(inputs["idx"])
    tok_emb, pos_emb = f(inputs["tok_emb"]), f(inputs["pos_emb"])
    xfull = (tok_emb[idx.reshape(-1)]
             + np.tile(pos_emb[:T], (B, 1))).astype(np.float32)  # [4096, D]
    wq, wk, wv = f(inputs["wq"]), f(inputs["wk"]), f(inputs["wv"])
    wlm_f, blm_f = f(inputs["wlm"]), f(inputs["blm"])

    shared = dict(
        wproj=f(inputs["wproj"]).astype(BF), bproj=f(inputs["bproj"]),
        ln1g=f(inputs["ln1_g"]), ln1b=f(inputs["ln1_b"]),
        ln2g=f(inputs["ln2_g"]), ln2b=f(inputs["ln2_b"]),
        w1=f(inputs["w1"]).astype(BF), b1=f(inputs["b1"]),
        w2=f(inputs["w2"]).astype(BF), b2=f(inputs["b2"]),
        lnfg=f(inputs["lnf_g"]).reshape(1, D),
        lnfb=f(inputs["lnf_b"]).reshape(1, D),
    )
    in_maps = []
    for c in range(NC):
        h0 = HL * c
        wqkv_c = np.concatenate(
            [w[:, h0:h0 + HL].transpose(0, 2, 1, 3).reshape(L, D, HL * HD)
             for w in (wq, wk, wv)], axis=2)  # [L, D, 384]
        m = dict(shared)
        m["x0"] = np.ascontiguousarray(xfull[c * TS:(c + 1) * TS])
        m["wqkv"] = np.ascontiguousarray(wqkv_c.astype(BF))
        m["wlm"] = np.ascontiguousarray(wlm_f[:, c * VC:(c + 1) * VC].astype(BF))
        m["blm"] = np.ascontiguousarray(blm_f[c * VC:(c + 1) * VC].reshape(1, VC))
        in_maps.append(m)
    return in_maps


def kernel(**inputs):
    nc = build_program()
    in_maps = _prep_inputs(inputs)
    res = run_bass_kernel_spmd(nc, in_maps, core_ids=list(range(NC)))
    logits = np.concatenate([res.results[c]["logits"] for c in range(NC)],
                            axis=1)  # [4096, 32000]
    target = np.asarray(inputs["target"]).reshape(-1)
    mx = logits.max(axis=-1)
    lse = mx + np.log(np.exp(logits - mx[:, None]).sum(axis=-1))
    tl = logits[np.arange(TT), target]
    loss = np.float32(np.mean(lse - tl))
    return logits.reshape(B, T, V), loss
